# revision 1
# baseline (speedup 1.0000x reference)
import sys
import numpy as np

for p in ("/opt/trn_rl_repo",):
    if p not in sys.path:
        sys.path.insert(0, p)

NC_CAP, DC, ROUT, EPS = 16, 32, 3, 1e-7
B, S, DIN, O = 256, 512, 256, 512     # full problem; O = NC_CAP*DC
NCORES = 8
BPC = B // NCORES                     # 32 batches per core
G = 8                                 # batches per pipelined group
NG = BPC // G
SC, IC, OC = S // 128, DIN // 128, O // 128   # 4, 2, 4 chunks

LAST_RESULTS = None
_NC_CACHE = None


def _kernel_numpy(u_vecs, W):
    u = u_vecs.astype(np.float32)
    w = W[0].astype(np.float32)
    uh = np.einsum('bsi,io->bso', u, w)
    uh = uh.reshape(B, S, NC_CAP, DC).transpose(0, 2, 1, 3)
    b = np.zeros((B, NC_CAP, S), dtype=np.float32)
    out = None
    for i in range(ROUT):
        m = b.max(axis=1, keepdims=True)
        e = np.exp(b - m)
        c = e / e.sum(axis=1, keepdims=True)
        o = np.einsum('bni,bnid->bnd', c, uh)
        out = o / np.sqrt((o * o).sum(-1, keepdims=True) + EPS)
        if i < ROUT - 1:
            b = np.einsum('bnd,bnid->bni', out, uh)
    return out.astype(np.float32)


def _build_bass(reps=1, internal_u=False, only_phase0=False, ngroups=None, window=2, tail_split=False, gplan=None, m1_accum=False):
    import concourse.bass as bass
    import concourse.tile as tile
    from concourse import mybir, bacc
    from contextlib import ExitStack

    f32, bf16 = mybir.dt.float32, mybir.dt.bfloat16
    AF = mybir.ActivationFunctionType
    ALU = mybir.AluOpType
    AX = mybir.AxisListType

    nc = bacc.Bacc()
    if internal_u:
        u_d = nc.dram_tensor("u", [BPC, S, DIN], f32)
    else:
        u_d = nc.declare_dram_parameter("u", [BPC, S, DIN], f32, isOutput=False)
    w_d = nc.declare_dram_parameter("W", [1, DIN, O], f32, isOutput=False)
    out_d = nc.declare_dram_parameter("out", [BPC, NC_CAP, DC], f32, isOutput=True)

    with ExitStack() as ctx:
        tc = ctx.enter_context(tile.TileContext(nc))
        const = ctx.enter_context(tc.tile_pool(name="const", bufs=1))
        sb_uf = ctx.enter_context(tc.tile_pool(name="sb_uf", bufs=6))
        sb_ub = ctx.enter_context(tc.tile_pool(name="sb_ub", bufs=4))
        sb_ut = ctx.enter_context(tc.tile_pool(name="sb_ut", bufs=4))
        sb_m = ctx.enter_context(tc.tile_pool(name="sb_m", bufs=4))
        sb_a = ctx.enter_context(tc.tile_pool(name="sb_a", bufs=4))
        sb_b = ctx.enter_context(tc.tile_pool(name="sb_b", bufs=4))
        # PSUM: 8 banks of 2KB/partition, whole-bank allocation per tag*buf
        ps_ut = ctx.enter_context(tc.tile_pool(name="ps_ut", bufs=2, space="PSUM"))
        ps_mq = ctx.enter_context(tc.tile_pool(name="ps_mq", bufs=2, space="PSUM"))
        ps_pr = ctx.enter_context(tc.tile_pool(name="ps_pr", bufs=2, space="PSUM"))
        ps_bt = ctx.enter_context(tc.tile_pool(name="ps_bt", bufs=2, space="PSUM"))

        # ---------------- constants ----------------
        ones = const.tile([128, 512], bf16, tag="ones")
        nc.gpsimd.memset(ones[:], 1.0)
        ident = const.tile([128, 128], bf16, tag="ident")
        nc.gpsimd.affine_select(ident[:], ones[:, 0:128], pattern=[[-1, 128]],
                                compare_op=ALU.is_equal, fill=0.0,
                                base=0, channel_multiplier=1)
        ident_f = const.tile([128, 128], f32, tag="identf")
        nc.vector.tensor_copy(ident_f[:], ident[:])

        # msel[:, c, n'] = 1 iff capsule(128c+p) == n'   (capsule = nd >> 5)
        msel = const.tile([128, OC, 16], bf16, tag="msel")
        for c in range(OC):
            t0 = const.tile([128, 16], bf16, tag=f"mselt{c}")
            nc.gpsimd.affine_select(t0[:], ones[:, 0:16], pattern=[[-32, 16]],
                                    compare_op=ALU.is_ge, fill=0.0,
                                    base=128 * c, channel_multiplier=1)
            nc.gpsimd.affine_select(msel[:, c, :], t0[:], pattern=[[32, 16]],
                                    compare_op=ALU.is_ge, fill=0.0,
                                    base=31 - 128 * c, channel_multiplier=-1)

        # BmaskT[n', nd] = 1 iff capsule(nd) == n'   ([16, 512])
        bmT = const.tile([16, 512], bf16, tag="bmT")
        for c in range(OC):
            tt = const.tile([16, 128], bf16, tag=f"bmTt{c}")
            nc.gpsimd.affine_select(tt[:], ones[0:16, 0:128], pattern=[[1, 128]],
                                    compare_op=ALU.is_ge, fill=0.0,
                                    base=128 * c, channel_multiplier=-32)
            nc.gpsimd.affine_select(bmT[:, 128 * c:128 * (c + 1)], tt[:],
                                    pattern=[[-1, 128]],
                                    compare_op=ALU.is_ge, fill=0.0,
                                    base=31 - 128 * c, channel_multiplier=32)

        eps_t = const.tile([16, 1], f32, tag="eps")
        nc.gpsimd.memset(eps_t[:], EPS)
        o16 = const.tile([128, 1], bf16, tag="o16")
        nc.gpsimd.memset(o16[:], 1.0 / NC_CAP)
        m4t = const.tile([128, 4], bf16, tag="m4t")
        nc.gpsimd.affine_select(m4t[:], ones[:, 0:4], pattern=[[-32, 4]],
                                compare_op=ALU.is_ge, fill=0.0,
                                base=0, channel_multiplier=1)
        msel4 = const.tile([128, 4], bf16, tag="msel4")
        nc.gpsimd.affine_select(msel4[:], m4t[:], pattern=[[32, 4]],
                                compare_op=ALU.is_ge, fill=0.0,
                                base=31, channel_multiplier=-1)

        # Pre-load act table set 6 (natural_log_exp_and_others): covers
        # copy/square/ln/exp so no reloads are needed anywhere in the kernel.
        nc.scalar.add_instruction(mybir.InstLoadActFuncSet(
            name=nc.get_next_instruction_name(), act_func_set_id=6,
            ins=[], outs=[]))

        # ---------------- W load, cast, transpose ----------------
        wf = const.tile([128, IC, O], f32, tag="wf")
        nc.sync.dma_start(wf[:], w_d[0].rearrange("(c p) o -> p c o", p=128))
        wn = const.tile([128, IC, O], bf16, tag="wn")        # W  [i-part, ic, nd]
        nc.scalar.copy(wn[:], wf[:])
        wT = const.tile([128, OC, IC, 128], bf16, tag="wT")  # W^T [nd-part, c, ic, i]
        pw0 = ps_mq.tile([128, 2 * IC, G, 16], f32, tag="mq", name="pw0")
        pwv = pw0.rearrange("p a b c -> p (a b c)").bitcast(bf16)
        for c in range(OC):
            for ic in range(IC):
                k = 2 * c + ic
                nc.tensor.transpose(pwv[:, 128 * k:128 * (k + 1)],
                                    wn[:, ic, 128 * c:128 * (c + 1)], ident[:])
                nc.scalar.copy(wT[:, c, ic, :], pwv[:, 128 * k:128 * (k + 1)])

        _ng = NG if ngroups is None else ngroups

        def group_tasks(g0, gg=G):
            """List of emit-thunks for one group of gg batches starting at g0."""
            st = {}

            def t_alloc():
                st['ub'] = sb_ub.tile([128, SC, G, DIN], bf16, tag="ub", name="ub")
                st['ut'] = [sb_ut.tile([128, G, S], bf16, tag=f"ut{ic}", name=f"ut{ic}")
                            for ic in range(IC)]
                if m1_accum:
                    st['m1f'] = sb_m.tile([128, G, IC], f32, tag="m1f", name="m1f")
            tasks = [t_alloc]

            def t_ph0(g):
                ub, ut = st['ub'], st['ut']
                uf = sb_uf.tile([128, SC, DIN], f32, tag="uf", name="uf")
                nc.sync.dma_start(uf[:], u_d[g0 + g].rearrange("(sc p) i -> p sc i", p=128))
                nc.gpsimd.tensor_copy(ub[:, 0:3, g, :], uf[:, 0:3, :])
                nc.vector.tensor_copy(ub[:, 3, g, :], uf[:, 3, :])
                pt = ps_ut.tile([128, IC, S], bf16, tag="utp", name="pt")
                for ic in range(IC):
                    for sc in range(SC):
                        nc.tensor.transpose(pt[:, ic, 128 * sc:128 * (sc + 1)],
                                            ub[:, sc, g, 128 * ic:128 * (ic + 1)],
                                            ident[:])
                if m1_accum:
                    m1f = st['m1f']
                    nc.vector.tensor_tensor_reduce(
                        ut[0][:, g, :], pt[:, 0, :], ones[:],
                        scale=1.0, scalar=0.0, op0=ALU.mult, op1=ALU.add,
                        accum_out=m1f[:, g:g + 1, 0])
                    nc.scalar.activation(ut[1][:, g, :], pt[:, 1, :], AF.Copy,
                                         accum_out=m1f[:, g:g + 1, 1])
                else:
                    nc.vector.tensor_copy(ut[0][:, g, :], pt[:, 0, :])
                    nc.scalar.copy(ut[1][:, g, :], pt[:, 1, :])
            for g in range(gg):
                tasks.append(lambda g=g: t_ph0(g))

            def t_m1():
                mq0 = ps_mq.tile([128, 2 * IC, G, 16], f32, tag="mq", name="mq0")
                st['mq0'] = mq0
                if m1_accum:
                    m1b = sb_m.tile([128, G, IC], bf16, tag="m1b", name="m1b")
                    nc.scalar.mul(m1b[:, 0:gg, :], st['m1f'][:, 0:gg, :], 1.0 / NC_CAP)
                else:
                    ub = st['ub']
                    # iter-1 m: column mean (1/16) sum_s u via PE into mq0[:,2]
                    for g in range(gg):
                        for ic in range(IC):
                            for sc in range(SC):
                                nc.tensor.matmul(mq0[:, 2, g, ic:ic + 1],
                                                 ub[:, sc, g, 128 * ic:128 * (ic + 1)],
                                                 o16[:],
                                                 start=(sc == 0), stop=(sc == SC - 1))
                    m1b = sb_m.tile([128, G, IC], bf16, tag="m1b", name="m1b")
                    nc.scalar.copy(m1b[:, 0:gg, :], mq0[:, 2, 0:gg, 0:IC])
                st['m1'] = [m1b[:, 0:gg, ic] for ic in range(IC)]
            tasks.append(t_m1)

            if only_phase0:
                return tasks

            def t_m(rt):
                if rt == 0:
                    st['mq'] = st['mq0']
                    st['mt'] = st['m1']
                    st['nfree'] = 1
                    return
                mq = ps_mq.tile([128, 2 * IC, G, 16], f32, tag="mq", name="mq")
                st['mq'] = mq
                st['nfree'] = 4
                ub, cT = st['ub'], st['cT']
                for g in range(gg):
                    for ic in range(IC):
                        for sc in range(SC):
                            nc.tensor.matmul(
                                mq[:, ic, g, :],
                                ub[:, sc, g, 128 * ic:128 * (ic + 1)],
                                cT[:, sc, g, :],
                                start=(sc == 0), stop=(sc == SC - 1))
                mtt = sb_m.tile([128, IC, G, 16], bf16, tag="mtt", name="mtt")
                nc.vector.tensor_copy(mtt[:, :, 0:gg, :], mq[:, 0:IC, 0:gg, :])
                st['mt'] = [mtt[:, ic, 0:gg, :] for ic in range(IC)]

            def t_pr(rt):
                mt, nfree = st['mt'], st['nfree']
                pr = ps_pr.tile([128, OC, 128], f32, tag="pr", name="pr")
                st['pr'] = pr
                for c in range(OC):
                    for ic in range(IC):
                        nc.tensor.matmul(
                            pr[:, c, 0:gg * nfree],
                            wn[:, ic, 128 * c:128 * (c + 1)],
                            mt[ic][:, :, 4 * c:4 * (c + 1)]
                            if nfree > 1 else mt[ic][:, 0:gg],
                            start=(ic == 0), stop=(ic == IC - 1))

            def t_ex(rt):
                pr = st['pr']
                if rt == 0:
                    o_sb = sb_a.tile([128, OC, G], f32, tag="o_sb0", name="o_sb0")
                    nc.vector.tensor_copy(o_sb[:, :, 0:gg], pr[:, :, 0:gg])
                else:
                    tm = sb_a.tile([128, OC, G, 4], bf16, tag="tm", name="tm")
                    nc.vector.tensor_tensor(
                        tm[:, :, 0:gg, :],
                        pr[:, :, 0:gg * 4].rearrange("p c (g n) -> p c g n", g=gg),
                        msel4.unsqueeze(1).unsqueeze(2).broadcast_to((128, OC, gg, 4)),
                        op=ALU.mult)
                    o_sb = sb_a.tile([128, OC, G], f32, tag="o_sb", name="o_sb")
                    nc.vector.reduce_sum(o_sb[:, :, 0:gg], tm[:, :, 0:gg, :], axis=AX.X)
                st['o_f'] = o_sb[:, :, 0:gg]

            def t_sq(rt):
                o_f, mq = st['o_f'], st['mq']
                sq = sb_a.tile([128, OC, G], bf16, tag="sq", name="sq")
                nc.vector.tensor_tensor(sq[:, :, 0:gg], o_f, o_f, op=ALU.mult)
                nrm = mq[0:16, 0].rearrange("p g n -> p (g n)")[:, 0:gg]
                for c in range(OC):
                    nc.tensor.matmul(nrm, msel[:, c, :], sq[:, c, 0:gg],
                                     start=(c == 0), stop=(c == OC - 1))
                lnx = sb_a.tile([16, G], f32, tag="lnx", name="lnx")
                nc.scalar.activation(lnx[:, 0:gg], nrm, AF.Ln, bias=eps_t[:])
                rsb = sb_a.tile([16, G], bf16, tag="rsb", name="rsb")
                nc.scalar.activation(rsb[:, 0:gg], lnx[:, 0:gg], AF.Exp, scale=-0.5)
                sbc = mq[:, 1].rearrange("p g n -> p (g n)")[:, 0:OC * gg].rearrange(
                    "p (a b) -> p a b", a=OC)
                for c in range(OC):
                    nc.tensor.matmul(sbc[:, c, :], bmT[:, 128 * c:128 * (c + 1)],
                                     rsb[:, 0:gg], start=True, stop=True)
                st['sbc'] = sbc

            def t_out(rt):
                o_f, sbc, pr = st['o_f'], st['sbc'], st['pr']
                ocf = sb_a.tile([128, OC, G], f32, tag="ocf", name="ocf")
                nc.vector.tensor_tensor(ocf[:, :, 0:gg], o_f, sbc, op=ALU.mult)
                for c in range(OC):
                    nc.tensor.transpose(pr[0:gg, c, :], ocf[:, c, 0:gg], ident_f[:])
                fout = sb_a.tile([G, OC * 128], f32, tag="fout", name="fout")
                nc.scalar.copy(fout[0:gg], pr[0:gg].rearrange("g c p -> g (c p)"))
                nc.sync.dma_start(
                    out_d[g0:g0 + gg].rearrange("g n d -> g (n d)"), fout[0:gg])

            def t_E(rt):
                o_f, sbc = st['o_f'], st['sbc']
                ocb = sb_a.tile([128, OC, G], bf16, tag="ocb", name="ocb")
                nc.vector.tensor_tensor(ocb[:, :, 0:gg], o_f, sbc, op=ALU.mult)
                E = sb_b.tile([128, OC, G, 4], bf16, tag="E", name="E")
                nc.vector.tensor_tensor(
                    E[:, :, 0:gg, :],
                    ocb[:, :, 0:gg].unsqueeze(3).broadcast_to((128, OC, gg, 4)),
                    msel4.unsqueeze(1).unsqueeze(2).broadcast_to((128, OC, gg, 4)),
                    op=ALU.mult)
                st['E'] = E

            def t_q(rt):
                # q region layout: [c, g, j] (contiguous [128,32] per (ic,c))
                E, mq = st['E'], st['mq']
                for ic in range(IC):
                    qv = mq[:, IC + ic].rearrange("p a b -> p (a b)")
                    for c in range(OC):
                        nc.tensor.matmul(
                            qv[:, 32 * c:32 * c + 4 * gg],
                            wT[:, c, ic, :],
                            E[:, c, 0:gg, :].rearrange("p a b -> p (a b)"),
                            start=True, stop=True)
                qb = sb_b.tile([128, IC, OC, G, 4], bf16, tag="qb", name="qb")
                nc.vector.tensor_copy(qb.rearrange("p a b c d -> p (a b c d)"),
                                      mq[:, IC:2 * IC].rearrange("p a b c -> p (a b c)"))
                st['qb'] = qb

            def t_bt(rt):
                qb, ut = st['qb'], st['ut']
                bt = ps_bt.tile([128, SC, G, 16], f32, tag="bt", name="bt")
                st['bt'] = bt
                for g in range(gg):
                    for sc in range(SC):
                        for ic in range(IC):
                            nc.tensor.matmul(bt[:, sc, g, :],
                                             ut[ic][:, g, 128 * sc:128 * (sc + 1)],
                                             qb[:, ic, :, g, :],
                                             start=(ic == 0), stop=(ic == IC - 1))

            def t_sm(rt):
                bt = st['bt']
                e_ = sb_b.tile([128, SC, G, 16], bf16, tag="e_", name="e_")
                nc.scalar.activation(e_[:, :, 0:gg, :], bt[:, :, 0:gg, :], AF.Exp)
                z_ = sb_b.tile([128, SC, G], f32, tag="z_", name="z_")
                nc.vector.reduce_sum(z_[:, :, 0:gg], e_[:, :, 0:gg, :], axis=AX.X)
                rz = sb_b.tile([128, SC, G], f32, tag="rz", name="rz")
                nc.vector.reciprocal(rz[:, :, 0:gg], z_[:, :, 0:gg])
                cT = sb_b.tile([128, SC, G, 16], bf16, tag="cT", name="cT")
                nc.vector.tensor_tensor(
                    cT[:, :, 0:gg, :], e_[:, :, 0:gg, :],
                    rz[:, :, 0:gg].unsqueeze(3).broadcast_to((128, SC, gg, 16)),
                    op=ALU.mult)
                st['cT'] = cT

            for rt in range(ROUT):
                tasks.append(lambda rt=rt: t_m(rt))
                tasks.append(lambda rt=rt: t_pr(rt))
                tasks.append(lambda rt=rt: t_ex(rt))
                tasks.append(lambda rt=rt: t_sq(rt))
                if rt == ROUT - 1:
                    tasks.append(lambda rt=rt: t_out(rt))
                else:
                    tasks.append(lambda rt=rt: t_E(rt))
                    tasks.append(lambda rt=rt: t_q(rt))
                    tasks.append(lambda rt=rt: t_bt(rt))
                    tasks.append(lambda rt=rt: t_sm(rt))
            return tasks

        # Software-pipelined emission. schedule:
        #  - "rr": round-robin sliding window of whole-group task lists
        #  - "shift": interleave group g's routing iters with group g+1's
        #    phase-0 (both dependency-ready at the same time)
        if gplan is not None:
            plan = list(gplan)
        else:
            plan = []
            for gi in range(_ng):
                if tail_split and gi == _ng - 1:
                    plan.append((gi * G, G // 2))
                    plan.append((gi * G + G // 2, G // 2))
                else:
                    plan.append((gi * G, G))
        plan = plan * reps
        all_tasks = [group_tasks(p0, pg) for (p0, pg) in plan]
        NPH = 2 + G  # alloc + per-batch phase0 + m1 marker below
        if window == 0:   # "shift" schedule
            nall = len(all_tasks)
            ph = [t[:G + 2] for t in all_tasks]   # alloc, 8x ph0, m1
            it = [t[G + 2:] for t in all_tasks]
            for t in ph[0]:
                t()
            for gi in range(nall):
                a = it[gi]
                b = ph[gi + 1] if gi + 1 < nall else []
                na, nbb = len(a), len(b)
                ia = ib = 0
                # interleave proportionally
                while ia < na or ib < nbb:
                    if ia < na:
                        a[ia](); ia += 1
                    if ib < nbb and (ia * nbb >= (ib + 1) * na or ia >= na):
                        b[ib](); ib += 1
        else:
            pos = [0] * len(all_tasks)
            head = 0
            while head < len(all_tasks):
                advanced = False
                for gi in range(head, min(head + window, len(all_tasks))):
                    if pos[gi] < len(all_tasks[gi]):
                        all_tasks[gi][pos[gi]]()
                        pos[gi] += 1
                        advanced = True
                while head < len(all_tasks) and pos[head] >= len(all_tasks[head]):
                    head += 1
                if not advanced and head < len(all_tasks):
                    all_tasks[head][pos[head]]()
                    pos[head] += 1
    nc.finalize()
    return nc


def kernel(u_vecs, W):
    global LAST_RESULTS, _NC_CACHE
    try:
        from concourse.bass_utils import run_bass_kernel_spmd
        if _NC_CACHE is None:
            _NC_CACHE = _build_bass()
        nc = _NC_CACHE
        u = np.ascontiguousarray(u_vecs, dtype=np.float32)
        w = np.ascontiguousarray(W, dtype=np.float32)
        in_maps = [{"u": u[c * BPC:(c + 1) * BPC], "W": w} for c in range(NCORES)]
        res = run_bass_kernel_spmd(nc, in_maps, core_ids=list(range(NCORES)))
        LAST_RESULTS = res
        out = np.concatenate([res.results[c]["out"] for c in range(NCORES)], axis=0)
        return out.astype(np.float32)
    except Exception as ex:
        import traceback
        traceback.print_exc()
        sys.stderr.write(f"[kernel.py] bass path failed ({ex!r}); numpy fallback\n")
        return _kernel_numpy(u_vecs, W)

